# revision 2
# baseline (speedup 1.0000x reference)
"""Causal self-attention (B=4, T=2048, C=1024, H=16, D=64) on 8 trn2 NeuronCores.

Sharding: core c = (batch b = c//2, head-group g = c%2). Megatron-style within a
batch: each core computes 8 heads' q/k/v (column-parallel) and a row-parallel
partial out-projection. Host sums the two partials per batch and adds the
rank-1 bias term (bo + bv @ wo) -- valid because softmax rows sum to 1, so v's
bias never needs to enter the kernel.

Per-core kernel (all matmuls bf16, fp32 PSUM accumulation):
  phase 1: qT,kT = (x@w)^T via lhsT=w, rhs=x^T (host pre-transposes x);
           v natural via lhsT=x^T-chunk, rhs=wv; ones column appended to v.
  phase 2: flash-style streaming attention in S^T orientation:
           S^T[k,q] = kT.T @ qT (head pairs packed in PE row groups 0/64),
           P^T = exp(S^T) (ScalarE, scale folded into q), causal masking by
           mask-multiply on diagonal tiles only; O^T accumulated via
           lhsT=v_tile (stationary), rhs=P^T; the ones column of v makes
           PSUM row 64 the softmax denominator Z for free.
  phase 3: y = O @ wo via lhsT=O^T (already the natural layout), rhs=wo.
"""
import numpy as np
import ml_dtypes

import concourse.tile as tile
from concourse import bacc, mybir
from concourse.bass_utils import run_bass_kernel_spmd

BF16 = ml_dtypes.bfloat16
F32 = mybir.dt.float32
BT16 = mybir.dt.bfloat16
AF = mybir.ActivationFunctionType
ALU = mybir.AluOpType

B, T, C, H, D = 4, 2048, 1024, 16, 64
G = 2              # head groups (cores per batch)
HL = H // G        # heads per core = 8
HD = HL * D        # local head dims = 512
NP = 4             # head pairs per core
NJQ = T // 512     # q chunks of 512 = 4
NIK = T // 128     # k tiles of 128 = 16
KC = C // 128      # contraction chunks = 8

_CACHED = {}


def _build():
    nc = bacc.Bacc("TRN2", debug=False)
    xT = nc.dram_tensor("xT", [C, T], BT16, kind="ExternalInput").ap()
    wq = nc.dram_tensor("wq", [C, HD], BT16, kind="ExternalInput").ap()
    wk = nc.dram_tensor("wk", [C, HD], BT16, kind="ExternalInput").ap()
    wv = nc.dram_tensor("wv", [C, HD], BT16, kind="ExternalInput").ap()
    wo = nc.dram_tensor("wo", [HD, C], BT16, kind="ExternalInput").ap()
    bq = nc.dram_tensor("bq", [128, NP], F32, kind="ExternalInput").ap()
    bk = nc.dram_tensor("bk", [128, NP], F32, kind="ExternalInput").ap()
    masks = nc.dram_tensor("masks", [128, 4, 512], BT16, kind="ExternalInput").ap()
    y = nc.dram_tensor("y", [T, C], F32, kind="ExternalOutput").ap()

    with tile.TileContext(nc) as tc:
        with (
            tc.tile_pool(name="consts", bufs=1) as consts,
            tc.tile_pool(name="xt", bufs=2) as xtp,
            tc.tile_pool(name="qk", bufs=1) as qkp,
            tc.tile_pool(name="vp", bufs=1) as vp,
            tc.tile_pool(name="otp", bufs=1) as otp,
            tc.tile_pool(name="pt", bufs=4) as ptp,
            tc.tile_pool(name="ptmp", bufs=2) as ptmpp,
            tc.tile_pool(name="zn", bufs=2) as znp,
            tc.tile_pool(name="yst", bufs=4) as ystp,
            tc.tile_pool(name="ps", bufs=2, space="PSUM") as ps,
        ):
            # ---- constants ----
            wq_sb = consts.tile([128, KC, HD], BT16, tag="wq")
            wk_sb = consts.tile([128, KC, HD], BT16, tag="wk")
            wv_sb = consts.tile([128, KC, HD], BT16, tag="wv")
            nc.sync.dma_start(wq_sb, wq.rearrange("(k p) c -> p k c", p=128))
            nc.sync.dma_start(wk_sb, wk.rearrange("(k p) c -> p k c", p=128))
            nc.sync.dma_start(wv_sb, wv.rearrange("(k p) c -> p k c", p=128))
            wo_sb = consts.tile([128, NP, C], BT16, tag="wo")
            nc.sync.dma_start(wo_sb, wo.rearrange("(t p) c -> p t c", p=128))
            masks_dma = consts.tile([128, 4, 512], BT16, tag="masks_dma")
            masks_sb = consts.tile([128, 4, 512], BT16, tag="masks")
            nc.sync.dma_start(masks_dma, masks)
            nc.vector.tensor_copy(masks_sb, masks_dma)
            bq_dma = consts.tile([128, NP], F32, tag="bq_dma")
            bq_sb = consts.tile([128, NP], F32, tag="bq")
            nc.sync.dma_start(bq_dma, bq)
            nc.vector.tensor_copy(bq_sb, bq_dma)
            bk_dma = consts.tile([128, NP], F32, tag="bk_dma")
            bk_sb = consts.tile([128, NP], F32, tag="bk")
            nc.sync.dma_start(bk_dma, bk)
            nc.vector.tensor_copy(bk_sb, bk_dma)
            ones64 = consts.tile([1, 64], BT16, tag="ones64")
            nc.vector.memset(ones64, 1.0)

            # ---- persistent activations ----
            qT = [qkp.tile([128, T], BT16, tag=f"qT{t}", name=f"qT{t}") for t in range(NP)]
            kT = [qkp.tile([128, T], BT16, tag=f"kT{t}", name=f"kT{t}") for t in range(NP)]
            v_sb = [vp.tile([128, HL * 65], BT16, tag=f"v{i}", name=f"v{i}") for i in range(NIK)]
            oT = [otp.tile([128, T], BT16, tag=f"oT{t}", name=f"oT{t}") for t in range(NP)]

            # ---- phase 1: projections ----
            for jt in range(NJQ):
                xt = xtp.tile([128, KC, 512], BT16, tag="xt")
                nc.sync.dma_start(
                    xt, xT[:, jt * 512:(jt + 1) * 512].rearrange("(k p) t -> p k t", p=128)
                )
                for t in range(NP):
                    p = ps.tile([128, 512], F32, tag="st")
                    for k in range(KC):
                        nc.tensor.matmul(
                            p, wq_sb[:, k, t * 128:(t + 1) * 128], xt[:, k, :],
                            start=(k == 0), stop=(k == KC - 1),
                        )
                    nc.vector.tensor_scalar(
                        qT[t][:, jt * 512:(jt + 1) * 512], p,
                        0.125, bq_sb[:, t:t + 1], ALU.mult, ALU.add,
                    )
                for t in range(NP):
                    p = ps.tile([128, 512], F32, tag="st")
                    for k in range(KC):
                        nc.tensor.matmul(
                            p, wk_sb[:, k, t * 128:(t + 1) * 128], xt[:, k, :],
                            start=(k == 0), stop=(k == KC - 1),
                        )
                    nc.vector.tensor_scalar_add(
                        kT[t][:, jt * 512:(jt + 1) * 512], p, bk_sb[:, t:t + 1]
                    )
                for s in range(4):
                    ik = jt * 4 + s
                    p = ps.tile([128, 512], F32, tag="st")
                    for k in range(KC):
                        nc.tensor.matmul(
                            p, xt[:, k, s * 128:(s + 1) * 128], wv_sb[:, k, :],
                            start=(k == 0), stop=(k == KC - 1),
                        )
                    vg = v_sb[ik].rearrange("p (h c) -> p h c", c=65)
                    nc.scalar.activation(
                        vg[:, :, 0:64], p.rearrange("p (h c) -> p h c", c=64), AF.Copy
                    )
                    nc.vector.memset(vg[:, :, 64:65], 1.0)

            # ---- phase 2: attention, head-pair at a time ----
            for t in range(NP):
                for jq in range(NJQ):
                    nik = 4 * jq + 4
                    qs = slice(jq * 512, (jq + 1) * 512)
                    o_ps = [ps.tile([65, 512], F32, tag="ot", bufs=3, name=f"ops{t}_{jq}_{_h}") for _h in range(2)]
                    pts = {}
                    for ik in range(nik):
                        st = ps.tile([128, 1024], F32, tag="st")
                        for hh in range(2):
                            r = slice(hh * 64, hh * 64 + 64)
                            nc.tensor.matmul(
                                st[:, hh * 512:(hh + 1) * 512],
                                kT[t][r, ik * 128:(ik + 1) * 128], qT[t][r, qs],
                                start=True, stop=True,
                            )
                        pt = ptp.tile([128, 1024], BT16, tag="pt")
                        d = ik - 4 * jq
                        if d >= 0:
                            ptm = ptmpp.tile([128, 1024], BT16, tag="ptmp")
                            nc.scalar.activation(ptm, st, AF.Exp)
                            for hh in range(2):
                                nc.vector.tensor_mul(
                                    pt[:, hh * 512:(hh + 1) * 512],
                                    ptm[:, hh * 512:(hh + 1) * 512],
                                    masks_sb[:, d, :],
                                )
                        else:
                            nc.scalar.activation(pt, st, AF.Exp)
                        pts[ik] = pt
                        # software-pipeline: AV for ik-1 after S^T/exp of ik
                        if ik > 0:
                            _av(nc, t, ik - 1, nik, v_sb, pts, o_ps)
                    _av(nc, t, nik - 1, nik, v_sb, pts, o_ps)
                    # normalization: Z is PSUM row 64 of each head's O^T
                    for hh in range(2):
                        zrow = znp.tile([65, 512], F32, tag="zrow")
                        nc.vector.tensor_copy(zrow[64:65, :], o_ps[hh][64:65, :])
                        z0 = znp.tile([1, 512], F32, tag="z0")
                        nc.sync.dma_start(z0, zrow[64:65, :])
                        rcp = znp.tile([1, 512], F32, tag="rcp")
                        nc.vector.reciprocal(rcp, z0)
                        rcp16 = znp.tile([1, 512], BT16, tag="rcp16")
                        nc.vector.tensor_copy(rcp16, rcp)
                        bc_ps = ps.tile([64, 512], F32, tag="bc", bufs=1)
                        nc.tensor.matmul(bc_ps, ones64, rcp16, start=True, stop=True)
                        bc_sb = znp.tile([64, 512], F32, tag="bc_sb")
                        nc.vector.tensor_copy(bc_sb, bc_ps)
                        if hh == 0:
                            nc.vector.tensor_mul(oT[t][0:64, qs], o_ps[hh][0:64, :], bc_sb)
                        else:
                            tmp = znp.tile([64, 512], BT16, tag="tmp_o")
                            nc.vector.tensor_mul(tmp, o_ps[hh][0:64, :], bc_sb)
                            nc.sync.dma_start(oT[t][64:128, qs], tmp)

            # ---- phase 3: out-projection ----
            for m in range(NIK):
                for n in range(2):
                    p = ps.tile([128, 512], F32, tag="st")
                    for t in range(NP):
                        nc.tensor.matmul(
                            p, oT[t][:, m * 128:(m + 1) * 128],
                            wo_sb[:, t, n * 512:(n + 1) * 512],
                            start=(t == 0), stop=(t == NP - 1),
                        )
                    ys = ystp.tile([128, 512], F32, tag="y")
                    nc.vector.tensor_copy(ys, p)
                    nc.sync.dma_start(y[m * 128:(m + 1) * 128, n * 512:(n + 1) * 512], ys)

    nc.compile()
    return nc


def _av(nc, t, ik, nik, v_sb, pts, o_ps):
    pt = pts[ik]
    for hh in range(2):
        h = 2 * t + hh
        nc.tensor.matmul(
            o_ps[hh], v_sb[ik][:, h * 65:h * 65 + 65],
            pt[:, hh * 512:(hh + 1) * 512],
            start=(ik == 0), stop=(ik == nik - 1),
        )


def kernel(x, wq, bq, wk, bk, wv, bv, wo, bo):
    x = np.asarray(x, dtype=np.float32)
    wq = np.asarray(wq, dtype=np.float32)
    bq = np.asarray(bq, dtype=np.float32)
    wk = np.asarray(wk, dtype=np.float32)
    bk = np.asarray(bk, dtype=np.float32)
    wv = np.asarray(wv, dtype=np.float32)
    bv = np.asarray(bv, dtype=np.float32)
    wo = np.asarray(wo, dtype=np.float32)
    bo = np.asarray(bo, dtype=np.float32)

    if "nc" not in _CACHED:
        _CACHED["nc"] = _build()
    nc = _CACHED["nc"]

    # host-side shards
    masks_np = np.zeros((128, 4, 512), dtype=BF16)
    qn = np.arange(512)[None, :]
    kn = np.arange(128)[:, None]
    for d in range(4):
        masks_np[:, d, :] = (qn >= kn + 128 * d).astype(BF16)

    per_g = []
    for g in range(G):
        cs = slice(g * HD, (g + 1) * HD)
        per_g.append({
            "wq": np.ascontiguousarray(wq[:, cs]).astype(BF16),
            "wk": np.ascontiguousarray(wk[:, cs]).astype(BF16),
            "wv": np.ascontiguousarray(wv[:, cs]).astype(BF16),
            "wo": np.ascontiguousarray(wo[cs, :]).astype(BF16),
            "bq": np.ascontiguousarray((bq[cs] / 8.0).reshape(NP, 128).T).astype(np.float32),
            "bk": np.ascontiguousarray(bk[cs].reshape(NP, 128).T).astype(np.float32),
            "masks": masks_np,
        })
    in_maps = []
    for c in range(8):
        b, g = divmod(c, G)
        m = dict(per_g[g])
        m["xT"] = np.ascontiguousarray(x[b].T).astype(BF16)
        in_maps.append(m)

    res = run_bass_kernel_spmd(nc, in_maps, core_ids=list(range(8)))

    const_row = (bo.astype(np.float64) + bv.astype(np.float64) @ wo.astype(np.float64))
    out = np.empty((B, T, C), dtype=np.float32)
    for b in range(B):
        acc = res.results[2 * b]["y"].astype(np.float64)
        acc += res.results[2 * b + 1]["y"]
        acc += const_row[None, :]
        out[b] = acc.astype(np.float32)
    return out
